# revision 24
# baseline (speedup 1.0000x reference)
"""Non-local (self-attention) block kernel for Trainium2, 8 NeuronCores.

Reference computation (per batch b):
    q = wq @ x + bq; k = wk @ x + bk; v = wv @ x + bv      (1x1 convs, x: [C, HW])
    attn = softmax_j(q^T q ... scores[i,j] = q[:,i].k[:,j])
    sa[d,i] = sum_j v[d,j] attn[i,j]
    out = gamma * (wsa @ sa + bsa) + x

Sharding: 8 cores = 4 batches x 2 halves of the i (query) dimension.
Each core gets the full x[b] (columns rotated so its own i-half sits at
columns [0, 2048)) and computes out[:, 0:2048] for that rotation.

Core algorithm (transposed-scores flash attention, no transposes needed):
    scoresT[j, i] = sum_d k[d, j] q[d, i]      (j on partitions, 128 per block)
    expT = exp(scoresT - SHIFT)                 (SHIFT global const, softmax-invariant)
    U[d', i] = sum_j vT'[j, d'] expT[j, i]      vT' = [v^T | ones]  -> row 64 = denom
    sa = U[0:64] / U[64]; out = wsa @ sa + bsa; final = gamma*out + x
"""

import numpy as np
from contextlib import ExitStack

import concourse.bass as bass
import concourse.bacc as bacc
import concourse.mybir as mybir
import concourse.tile as tile
from concourse.bass_utils import run_bass_kernel_spmd
from concourse.masks import make_identity

F32 = mybir.dt.float32
BF16 = mybir.dt.bfloat16
AF = mybir.ActivationFunctionType
OP = mybir.AluOpType

B, C, HW, D = 4, 256, 4096, 64
IH = 2048            # i-rows computed per core
ICH = 512            # i-chunk (matmul moving dim)
NIC = IH // ICH      # 4
JBS = 128            # j-block (scores partition block)
NJB = HW // JBS      # 32
GJ = 3               # j-blocks per exp group (3 PSUM banks per scores tile)
SHIFT = 30.0         # global constant subtracted before exp (softmax-invariant)

N_CORES = 8


def build_program(reps=1):
    """Build the SPMD program. reps>1 wraps the main attention loop in a
    hardware For loop (identical recomputation) — used only for timing."""
    nc = bacc.Bacc("TRN2", target_bir_lowering=False, debug=False)

    x_d = nc.declare_dram_parameter("x", [C, HW], F32, isOutput=False)
    wq_d = nc.declare_dram_parameter("wq", [D, C], F32, isOutput=False)
    wk_d = nc.declare_dram_parameter("wk", [D, C], F32, isOutput=False)
    wv_d = nc.declare_dram_parameter("wv", [D, C], F32, isOutput=False)
    wsa_d = nc.declare_dram_parameter("wsa", [C, D], F32, isOutput=False)
    bq_d = nc.declare_dram_parameter("bq", [D, 1], F32, isOutput=False)
    bk_d = nc.declare_dram_parameter("bk", [D, 1], F32, isOutput=False)
    bv_d = nc.declare_dram_parameter("bv", [1, D], F32, isOutput=False)
    bsa_d = nc.declare_dram_parameter("bsa", [1, C], F32, isOutput=False)
    gam_d = nc.declare_dram_parameter("gamma", [1, 1], F32, isOutput=False)
    out_d = nc.declare_dram_parameter("out", [C, IH], F32, isOutput=True)

    with ExitStack() as ctx:
        tc = ctx.enter_context(tile.TileContext(nc))
        ps = ctx.enter_context(tc.tile_pool(name="ps", bufs=1))
        xt = ctx.enter_context(tc.tile_pool(name="xt", bufs=3))
        ep = ctx.enter_context(tc.tile_pool(name="ep", bufs=2))
        scp = ctx.enter_context(tc.tile_pool(name="scp", bufs=2, space="PSUM"))
        acc = ctx.enter_context(tc.tile_pool(name="acc", bufs=2, space="PSUM"))

        # --- warm the exp activation table early (overlaps with DMAs) ---
        warm = ps.tile([1, 8], F32, name="warm", tag="warm")
        nc.vector.memset(warm[:], 0.0)
        nc.scalar.activation(warm[:], warm[:], AF.Exp, bias=0.0)

        # --- weight / bias loads ---
        wq_sb = ps.tile([D, C], F32, name="wq_sb", tag="wq_sb")
        wk_sb = ps.tile([D, C], F32, name="wk_sb", tag="wk_sb")
        wv_sb = ps.tile([D, C], F32, name="wv_sb", tag="wv_sb")
        nc.sync.dma_start(wq_sb[:], wq_d[:])
        nc.sync.dma_start(wk_sb[:], wk_d[:])
        nc.sync.dma_start(wv_sb[:], wv_d[:])
        wsa_sb = [ps.tile([128, D], F32, name=f"wsa_sb{cb}", tag=f"wsa_sb{cb}") for cb in range(2)]
        for cb in range(2):
            nc.sync.dma_start(wsa_sb[cb][:], wsa_d[cb * 128:(cb + 1) * 128, :])
        # bq/bk stacked twice: partitions 0-63 and 64-127 (for row-pair packing)
        bqq = ps.tile([128, 1], F32, name="bqq", tag="bqq")
        bkk = ps.tile([128, 1], F32, name="bkk", tag="bkk")
        bv_row = ps.tile([1, D], F32, name="bv_row", tag="bv_row")
        bsa_row = ps.tile([1, C], F32, name="bsa_row", tag="bsa_row")
        gam_sb = ps.tile([1, 1], F32, name="gam_sb", tag="gam_sb")
        nc.sync.dma_start(bqq[0:D, :], bq_d[:])
        nc.sync.dma_start(bqq[D:2 * D, :], bq_d[:])
        nc.sync.dma_start(bkk[0:D, :], bk_d[:])
        nc.sync.dma_start(bkk[D:2 * D, :], bk_d[:])
        nc.sync.dma_start(bv_row[:], bv_d[:])
        nc.sync.dma_start(bsa_row[:], bsa_d[:])
        nc.sync.dma_start(gam_sb[:], gam_d[:])

        # broadcasts via K=1 ones-matmul (exact for fp32)
        ones128 = ps.tile([1, 128], F32, name="ones128", tag="ones128")
        nc.vector.memset(ones128[:], 1.0)
        # bf16 ones row at partition 64 (for the per-chunk recip broadcast)
        ones_bf65 = ps.tile([65, D], BF16, name="ones_bf65", tag="ones_bf65")
        nc.vector.memset(ones_bf65[D:D + 1, :], 1.0)
        bvB = ps.tile([128, D], F32, name="bvB", tag="bvB")
        gammaB = ps.tile([128, 1], F32, name="gammaB", tag="gammaB")
        bc_ps = acc.tile([128, ICH], F32, name="bc_ps", tag="acc")
        nc.tensor.matmul(bc_ps[:, 0:D], ones128[:], bv_row[:],
                         start=True, stop=True)
        nc.vector.tensor_copy(bvB[:], bc_ps[:, 0:D])
        bc_ps2 = acc.tile([128, ICH], F32, name="bc_ps2", tag="acc")
        nc.tensor.matmul(bc_ps2[:, 0:1], ones128[:], gam_sb[:],
                         start=True, stop=True)
        nc.vector.tensor_copy(gammaB[:], bc_ps2[:, 0:1])

        # negative shift for exp (softmax-invariant constant)
        nshift = ps.tile([128, 1], F32, name="nshift", tag="nshift")
        nc.vector.memset(nshift[:], -SHIFT)

        # identities for PE transposes
        id64 = ps.tile([64, 64], F32, name="id64", tag="id64")
        make_identity(nc, id64[:])
        id128 = ps.tile([128, 128], F32, name="id128", tag="id128")
        make_identity(nc, id128[:])

        # --- x loads + bf16 casts ---
        xf = [[ps.tile([128, ICH], F32, name=f"xf_{cb}_{s}", tag=f"xf_{cb}_{s}") for s in range(8)]
              for cb in range(2)]
        xb = [[ps.tile([128, ICH], BF16, name=f"xb_{cb}_{s}", tag=f"xb_{cb}_{s}") for s in range(8)]
              for cb in range(2)]
        for s in range(8):
            for cb in range(2):
                nc.sync.dma_start(
                    xf[cb][s][:], x_d[cb * 128:(cb + 1) * 128, s * ICH:(s + 1) * ICH])
                nc.gpsimd.tensor_copy(xb[cb][s][:], xf[cb][s][:])

        # --- transpose weights: w^T chunks [128c, 64d] (bf16) ---
        wqT = [ps.tile([128, D], BF16, name=f"wqT{cb}", tag=f"wqT{cb}") for cb in range(2)]
        wkT = [ps.tile([128, D], BF16, name=f"wkT{cb}", tag=f"wkT{cb}") for cb in range(2)]
        wvT = [ps.tile([128, D], BF16, name=f"wvT{cb}", tag=f"wvT{cb}") for cb in range(2)]
        for w_sb, wT in ((wq_sb, wqT), (wk_sb, wkT), (wv_sb, wvT)):
            for cb in range(2):
                tr = acc.tile([128, ICH], F32, name="acc", tag="acc")
                nc.tensor.transpose(
                    tr[:, 0:D], w_sb[:, cb * 128:(cb + 1) * 128], id64[:])
                nc.vector.tensor_copy(wT[cb][:], tr[:, 0:D])

        # wsaT' = [wsa^T ; bsa] as [65, 256] bf16
        wsaT = ps.tile([65, C], BF16, name="wsaT", tag="wsaT")
        for cb in range(2):
            tr = acc.tile([128, ICH], F32, name="acc", tag="acc")
            nc.tensor.transpose(tr[0:D, 0:128], wsa_sb[cb][:], id128[:])
            nc.vector.tensor_copy(wsaT[0:D, cb * 128:(cb + 1) * 128], tr[0:D, 0:128])
        bsa_bf = ps.tile([1, C], BF16, name="bsa_bf", tag="bsa_bf")
        nc.vector.tensor_copy(bsa_bf[:], bsa_row[:])
        nc.sync.dma_start(wsaT[D:D + 1, :], bsa_bf[:])

        # --- projections ---
        # k and q are computed TWICE (partitions 0-63 and 64-127, via col-tiled
        # matmul pairs) so scores matmuls can row-pair-pack the PE array.
        # k: [128, HW] bf16 (both halves = k[d, j]), per 512-chunk
        kk = [ps.tile([128, ICH], BF16, name=f"kk{s}", tag=f"kk{s}") for s in range(8)]
        for s in range(8):
            kp = scp.tile([128, GJ * ICH], F32, name="sc", tag="sc")
            for cb in range(2):
                nc.tensor.matmul(kp[0:D, 0:ICH], wkT[cb][:], xb[cb][s][:],
                                 start=(cb == 0), stop=(cb == 1))
            for cb in range(2):
                nc.tensor.matmul(kp[D:2 * D, 0:ICH], wkT[cb][:], xb[cb][s][:],
                                 start=(cb == 0), stop=(cb == 1),
                                 tile_position=(0, 64))
            nc.vector.tensor_scalar(out=kk[s][:], in0=kp[:, 0:ICH],
                                    scalar1=bkk[:], scalar2=None, op0=OP.add)

        # q: [128, IH] bf16 (both halves), i-chunks 0..3
        qq = [ps.tile([128, ICH], BF16, name=f"qq{ic}", tag=f"qq{ic}") for ic in range(NIC)]

        def emit_qq(ic):
            qp = scp.tile([128, GJ * ICH], F32, name="sc", tag="sc")
            for cb in range(2):
                nc.tensor.matmul(qp[0:D, 0:ICH], wqT[cb][:], xb[cb][ic][:],
                                 start=(cb == 0), stop=(cb == 1))
            for cb in range(2):
                nc.tensor.matmul(qp[D:2 * D, 0:ICH], wqT[cb][:], xb[cb][ic][:],
                                 start=(cb == 0), stop=(cb == 1),
                                 tile_position=(0, 64))
            nc.vector.tensor_scalar(out=qq[ic][:], in0=qp[:, 0:ICH],
                                    scalar1=bqq[:], scalar2=None, op0=OP.add)

        # vT' = [v^T | ones]: [128j, 65] per j-block, packed [128, NJB*65] bf16
        vto = ps.tile([128, NJB * 65], BF16, name="vto", tag="vto")
        ones_ap = vto[:].rearrange("p (g e) -> p g e", e=65)[:, :, 64:65]
        nc.vector.memset(ones_ap, 1.0)

        def emit_vt(jb):
            s, col = jb // 4, (jb % 4) * 128
            vp = acc.tile([128, ICH], F32, name="acc", tag="acc")
            for cb in range(2):
                nc.tensor.matmul(vp[:, 0:D], xb[cb][s][:, col:col + 128], wvT[cb][:],
                                 start=(cb == 0), stop=(cb == 1))
            nc.vector.tensor_tensor(vto[:, jb * 65:jb * 65 + 64], vp[:, 0:D],
                                    bvB[:], op=OP.add)

        emit_qq(0)
        if reps > 1:
            # timing build: everything hoisted out of the repeat loop
            for ic in range(1, NIC):
                emit_qq(ic)
            for jb in range(NJB):
                emit_vt(jb)

        # --- main attention loop ---
        rep_ctx = tc.For_i(0, reps, 1) if reps > 1 else None
        if rep_ctx is not None:
            ctx.enter_context(rep_ctx)
        groups = [list(range(g, min(g + GJ, NJB))) for g in range(0, NJB, GJ)]
        for ic in range(NIC):
            U = acc.tile([128, ICH], F32, name="acc", tag="acc")
            for gi, jbs in enumerate(groups):
                sc = scp.tile([128, GJ * ICH], F32, name="sc", tag="sc")
                for t, jb in enumerate(jbs):
                    s, col = jb // 4, (jb % 4) * 128
                    rg = jb % 2  # alternate PE row groups -> pairs overlap
                    nc.tensor.matmul(sc[:, t * ICH:(t + 1) * ICH],
                                     kk[s][rg * D:(rg + 1) * D, col:col + 128],
                                     qq[ic][rg * D:(rg + 1) * D, :],
                                     start=True, stop=True)
                ex = xt.tile([128, GJ * ICH], BF16, name="xt", tag="xt")
                n = len(jbs) * ICH
                nc.scalar.activation(ex[:, 0:n], sc[:, 0:n], AF.Exp, bias=nshift[:])
                if ic == 0 and reps == 1:
                    # first pass: produce v^T blocks just-in-time (fills PE
                    # while ACT works) and later q chunks early
                    for jb in jbs:
                        emit_vt(jb)
                    if gi < NIC - 1:
                        emit_qq(gi + 1)
                for t, jb in enumerate(jbs):
                    nc.tensor.matmul(U[0:65, :], vto[:, jb * 65:jb * 65 + 65],
                                     ex[:, t * ICH:(t + 1) * ICH],
                                     start=(jb == 0), stop=(jb == NJB - 1))

            # epilogue: normalize, project, residual
            u_sb = ep.tile([65, ICH], F32, name="u_sb", tag="u_sb")
            nc.vector.tensor_copy(u_sb[:], U[0:65, :])
            rc65 = ep.tile([65, ICH], F32, name="rc65", tag="rc65")
            nc.vector.reciprocal(rc65[D:D + 1, :], u_sb[D:D + 1, :])
            rcb_bf = ep.tile([65, ICH], BF16, name="rcb_bf", tag="rcb_bf")
            nc.vector.tensor_copy(rcb_bf[D:D + 1, :], rc65[D:D + 1, :])
            rcb = acc.tile([128, ICH], F32, name="rcb", tag="acc")
            nc.tensor.matmul(rcb[0:D, :], ones_bf65[D:D + 1, 0:D],
                             rcb_bf[D:D + 1, :], start=True, stop=True)
            usb = ep.tile([65, ICH], BF16, name="usb", tag="usb")
            nc.vector.tensor_tensor(usb[0:D, :], u_sb[0:D, :], rcb[0:D, :],
                                    op=OP.mult)
            nc.vector.memset(usb[D:D + 1, :], 1.0)
            for cb in range(2):
                op_ps = acc.tile([128, ICH], F32, name="acc", tag="acc")
                nc.tensor.matmul(op_ps[:], wsaT[:, cb * 128:(cb + 1) * 128], usb[:],
                                 start=True, stop=True)
                fin = ep.tile([128, ICH], F32, name="fin", tag="fin")
                nc.vector.scalar_tensor_tensor(
                    out=fin[:], in0=op_ps[:], scalar=gammaB[:], in1=xf[cb][ic][:],
                    op0=OP.mult, op1=OP.add)
                nc.sync.dma_start(
                    out_d[cb * 128:(cb + 1) * 128, ic * ICH:(ic + 1) * ICH], fin[:])

    nc.compile()
    return nc


_CACHE = {}


def _get_program():
    if "nc" not in _CACHE:
        _CACHE["nc"] = build_program()
    return _CACHE["nc"]


def make_in_maps(inputs):
    x = np.asarray(inputs["x"], np.float32).reshape(B, C, HW)
    wq = np.ascontiguousarray(np.asarray(inputs["wq"], np.float32))
    wk = np.ascontiguousarray(np.asarray(inputs["wk"], np.float32))
    wv = np.ascontiguousarray(np.asarray(inputs["wv"], np.float32))
    wsa = np.ascontiguousarray(np.asarray(inputs["wsa"], np.float32))
    bq = np.asarray(inputs["bq"], np.float32).reshape(D, 1)
    bk = np.asarray(inputs["bk"], np.float32).reshape(D, 1)
    bv = np.asarray(inputs["bv"], np.float32).reshape(1, D)
    bsa = np.asarray(inputs["bsa"], np.float32).reshape(1, C)
    gamma = np.asarray(inputs["gamma"], np.float32).reshape(1, 1)

    in_maps = []
    for core in range(N_CORES):
        b, h = core // 2, core % 2
        if h == 0:
            xc = x[b]
        else:
            xc = np.concatenate([x[b][:, IH:], x[b][:, :IH]], axis=1)
        in_maps.append({
            "x": np.ascontiguousarray(xc),
            "wq": wq, "wk": wk, "wv": wv, "wsa": wsa,
            "bq": bq, "bk": bk, "bv": bv, "bsa": bsa, "gamma": gamma,
        })
    return in_maps


def assemble_output(results):
    out = np.empty((B, C, HW), np.float32)
    for core in range(N_CORES):
        b, h = core // 2, core % 2
        out[b][:, h * IH:(h + 1) * IH] = results[core]["out"]
    return out.reshape(B, C, 64, 64)


def kernel(**inputs):
    nc = _get_program()
    in_maps = make_in_maps(inputs)
    res = run_bass_kernel_spmd(nc, in_maps, core_ids=list(range(N_CORES)))
    return assemble_output(res.results)


# revision 29
# speedup vs baseline: 1.1556x; 1.1556x over previous
"""Non-local (self-attention) block kernel for Trainium2, 8 NeuronCores.

Reference computation (per batch b):
    q = wq @ x + bq; k = wk @ x + bk; v = wv @ x + bv      (1x1 convs, x: [C, HW])
    attn = softmax_j(q^T q ... scores[i,j] = q[:,i].k[:,j])
    sa[d,i] = sum_j v[d,j] attn[i,j]
    out = gamma * (wsa @ sa + bsa) + x

Sharding: 8 cores = 4 batches x 2 halves of the i (query) dimension.
Each core gets the full x[b] (columns rotated so its own i-half sits at
columns [0, 2048)) and computes out[:, 0:2048] for that rotation.

Core algorithm (transposed-scores flash attention, no transposes needed):
    scoresT[j, i] = sum_d k[d, j] q[d, i]      (j on partitions, 128 per block)
    expT = exp(scoresT - SHIFT)                 (SHIFT global const, softmax-invariant)
    U[d', i] = sum_j vT'[j, d'] expT[j, i]      vT' = [v^T | ones]  -> row 64 = denom
    sa = U[0:64] / U[64]; out = wsa @ sa + bsa; final = gamma*out + x
"""

import numpy as np
from contextlib import ExitStack

import concourse.bass as bass
import concourse.bacc as bacc
import concourse.mybir as mybir
import concourse.tile as tile
from concourse.bass_utils import run_bass_kernel_spmd
from concourse.masks import make_identity

F32 = mybir.dt.float32
BF16 = mybir.dt.bfloat16
AF = mybir.ActivationFunctionType
OP = mybir.AluOpType

B, C, HW, D = 4, 256, 4096, 64
IH = 2048            # i-rows computed per core
ICH = 512            # i-chunk (matmul moving dim)
NIC = IH // ICH      # 4
JBS = 128            # j-block (scores partition block)
NJB = HW // JBS      # 32
GJ = 3               # j-blocks per exp group (3 PSUM banks per scores tile)
SHIFT = 30.0         # global constant subtracted before exp (softmax-invariant)

N_CORES = 8


def build_program(reps=1, variant="full"):
    """Build the SPMD program. reps>1 wraps the main attention loop in a
    hardware For loop (identical recomputation) — used only for timing.
    variant: full | scores | scores_unpaired | expo | umm (microbenchmarks)."""
    nc = bacc.Bacc("TRN2", target_bir_lowering=False, debug=False)

    x_d = nc.declare_dram_parameter("x", [C, HW], F32, isOutput=False)
    wq_d = nc.declare_dram_parameter("wq", [D, C], F32, isOutput=False)
    wk_d = nc.declare_dram_parameter("wk", [D, C], F32, isOutput=False)
    wv_d = nc.declare_dram_parameter("wv", [D, C], F32, isOutput=False)
    wsa_d = nc.declare_dram_parameter("wsa", [C, D], F32, isOutput=False)
    bq_d = nc.declare_dram_parameter("bq", [D, 1], F32, isOutput=False)
    bk_d = nc.declare_dram_parameter("bk", [D, 1], F32, isOutput=False)
    bv_d = nc.declare_dram_parameter("bv", [1, D], F32, isOutput=False)
    bsa_d = nc.declare_dram_parameter("bsa", [1, C], F32, isOutput=False)
    gam_d = nc.declare_dram_parameter("gamma", [1, 1], F32, isOutput=False)
    out_d = nc.declare_dram_parameter("out", [C, IH], F32, isOutput=True)

    with ExitStack() as ctx:
        tc = ctx.enter_context(tile.TileContext(nc))
        ps = ctx.enter_context(tc.tile_pool(name="ps", bufs=1))
        xt = ctx.enter_context(tc.tile_pool(name="xt", bufs=4))
        ep = ctx.enter_context(tc.tile_pool(name="ep", bufs=2))
        scp = ctx.enter_context(tc.tile_pool(name="scp", bufs=2, space="PSUM"))
        acc = ctx.enter_context(tc.tile_pool(name="acc", bufs=2, space="PSUM"))

        # --- warm the exp activation table early (overlaps with DMAs) ---
        warm = ps.tile([1, 8], F32, name="warm", tag="warm")
        nc.vector.memset(warm[:], 0.0)
        nc.scalar.activation(warm[:], warm[:], AF.Exp, bias=0.0)

        # --- weight / bias loads ---
        wq_sb = ps.tile([D, C], F32, name="wq_sb", tag="wq_sb")
        wk_sb = ps.tile([D, C], F32, name="wk_sb", tag="wk_sb")
        wv_sb = ps.tile([D, C], F32, name="wv_sb", tag="wv_sb")
        nc.sync.dma_start(wq_sb[:], wq_d[:])
        nc.sync.dma_start(wk_sb[:], wk_d[:])
        nc.sync.dma_start(wv_sb[:], wv_d[:])
        wsa_sb = [ps.tile([128, D], F32, name=f"wsa_sb{cb}", tag=f"wsa_sb{cb}") for cb in range(2)]
        for cb in range(2):
            nc.sync.dma_start(wsa_sb[cb][:], wsa_d[cb * 128:(cb + 1) * 128, :])
        # bq/bk stacked twice: partitions 0-63 and 64-127 (for row-pair packing)
        bqq = ps.tile([128, 1], F32, name="bqq", tag="bqq")
        bkk = ps.tile([128, 1], F32, name="bkk", tag="bkk")
        bv_row = ps.tile([1, D], F32, name="bv_row", tag="bv_row")
        bsa_row = ps.tile([1, C], F32, name="bsa_row", tag="bsa_row")
        gam_sb = ps.tile([1, 1], F32, name="gam_sb", tag="gam_sb")
        nc.sync.dma_start(bqq[0:D, :], bq_d[:])
        nc.sync.dma_start(bqq[D:2 * D, :], bq_d[:])
        nc.sync.dma_start(bkk[0:D, :], bk_d[:])
        nc.sync.dma_start(bkk[D:2 * D, :], bk_d[:])
        nc.sync.dma_start(bv_row[:], bv_d[:])
        nc.sync.dma_start(bsa_row[:], bsa_d[:])
        nc.sync.dma_start(gam_sb[:], gam_d[:])

        # broadcasts via K=1 ones-matmul (exact for fp32)
        ones128 = ps.tile([1, 128], F32, name="ones128", tag="ones128")
        nc.vector.memset(ones128[:], 1.0)
        # bf16 ones row at partition 64 (for the per-chunk recip broadcast)
        ones_bf65 = ps.tile([65, D], BF16, name="ones_bf65", tag="ones_bf65")
        nc.vector.memset(ones_bf65[D:D + 1, :], 1.0)
        bvB = ps.tile([128, D], F32, name="bvB", tag="bvB")
        gammaB = ps.tile([128, 1], F32, name="gammaB", tag="gammaB")
        bc_ps = acc.tile([128, ICH], F32, name="bc_ps", tag="acc")
        nc.tensor.matmul(bc_ps[:, 0:D], ones128[:], bv_row[:],
                         start=True, stop=True)
        nc.vector.tensor_copy(bvB[:], bc_ps[:, 0:D])
        bc_ps2 = acc.tile([128, ICH], F32, name="bc_ps2", tag="acc")
        nc.tensor.matmul(bc_ps2[:, 0:1], ones128[:], gam_sb[:],
                         start=True, stop=True)
        nc.vector.tensor_copy(gammaB[:], bc_ps2[:, 0:1])

        # negative shift for exp (softmax-invariant constant)
        nshift = ps.tile([128, 1], F32, name="nshift", tag="nshift")
        nc.vector.memset(nshift[:], -SHIFT)

        # identities for PE transposes
        id64 = ps.tile([64, 64], F32, name="id64", tag="id64")
        make_identity(nc, id64[:])
        id128 = ps.tile([128, 128], F32, name="id128", tag="id128")
        make_identity(nc, id128[:])

        # --- x loads + bf16 casts ---
        xf = [[ps.tile([128, ICH], F32, name=f"xf_{cb}_{s}", tag=f"xf_{cb}_{s}") for s in range(8)]
              for cb in range(2)]
        xb = [[ps.tile([128, ICH], BF16, name=f"xb_{cb}_{s}", tag=f"xb_{cb}_{s}") for s in range(8)]
              for cb in range(2)]
        for s in range(8):
            for cb in range(2):
                nc.sync.dma_start(
                    xf[cb][s][:], x_d[cb * 128:(cb + 1) * 128, s * ICH:(s + 1) * ICH])
                nc.gpsimd.tensor_copy(xb[cb][s][:], xf[cb][s][:])

        # --- transpose weights: w^T chunks [128c, 64d] (bf16) ---
        wqT = [ps.tile([128, D], BF16, name=f"wqT{cb}", tag=f"wqT{cb}") for cb in range(2)]
        wkT = [ps.tile([128, D], BF16, name=f"wkT{cb}", tag=f"wkT{cb}") for cb in range(2)]
        wvT = [ps.tile([128, D], BF16, name=f"wvT{cb}", tag=f"wvT{cb}") for cb in range(2)]
        for w_sb, wT in ((wq_sb, wqT), (wk_sb, wkT), (wv_sb, wvT)):
            for cb in range(2):
                tr = acc.tile([128, ICH], F32, name="acc", tag="acc")
                nc.tensor.transpose(
                    tr[:, 0:D], w_sb[:, cb * 128:(cb + 1) * 128], id64[:])
                nc.vector.tensor_copy(wT[cb][:], tr[:, 0:D])

        # wsaT' = [wsa^T ; bsa] as [65, 256] bf16
        wsaT = ps.tile([65, C], BF16, name="wsaT", tag="wsaT")
        for cb in range(2):
            tr = acc.tile([128, ICH], F32, name="acc", tag="acc")
            nc.tensor.transpose(tr[0:D, 0:128], wsa_sb[cb][:], id128[:])
            nc.vector.tensor_copy(wsaT[0:D, cb * 128:(cb + 1) * 128], tr[0:D, 0:128])
        bsa_bf = ps.tile([1, C], BF16, name="bsa_bf", tag="bsa_bf")
        nc.vector.tensor_copy(bsa_bf[:], bsa_row[:])
        nc.sync.dma_start(wsaT[D:D + 1, :], bsa_bf[:])

        # --- projections ---
        # k and q are computed TWICE (partitions 0-63 and 64-127, via col-tiled
        # matmul pairs) so scores matmuls can row-pair-pack the PE array.
        # k: [128, HW] bf16 (both halves = k[d, j]), per 512-chunk
        kk = [ps.tile([128, ICH], BF16, name=f"kk{s}", tag=f"kk{s}") for s in range(8)]
        for s in range(8):
            kp = scp.tile([128, GJ * ICH], F32, name="sc", tag="sc")
            for cb in range(2):
                nc.tensor.matmul(kp[0:D, 0:ICH], wkT[cb][:], xb[cb][s][:],
                                 start=(cb == 0), stop=(cb == 1))
            for cb in range(2):
                nc.tensor.matmul(kp[D:2 * D, 0:ICH], wkT[cb][:], xb[cb][s][:],
                                 start=(cb == 0), stop=(cb == 1),
                                 tile_position=(0, 64))
            nc.vector.tensor_scalar(out=kk[s][:], in0=kp[:, 0:ICH],
                                    scalar1=bkk[:], scalar2=None, op0=OP.add)

        # q: [128, IH] bf16 (both halves), i-chunks 0..3
        qq = [ps.tile([128, ICH], BF16, name=f"qq{ic}", tag=f"qq{ic}") for ic in range(NIC)]

        def emit_qq(ic):
            qp = scp.tile([128, GJ * ICH], F32, name="sc", tag="sc")
            for cb in range(2):
                nc.tensor.matmul(qp[0:D, 0:ICH], wqT[cb][:], xb[cb][ic][:],
                                 start=(cb == 0), stop=(cb == 1))
            for cb in range(2):
                nc.tensor.matmul(qp[D:2 * D, 0:ICH], wqT[cb][:], xb[cb][ic][:],
                                 start=(cb == 0), stop=(cb == 1),
                                 tile_position=(0, 64))
            nc.vector.tensor_scalar(out=qq[ic][:], in0=qp[:, 0:ICH],
                                    scalar1=bqq[:], scalar2=None, op0=OP.add)

        # vT' = [v^T | ones]: [128j, 65] per j-block, packed [128, NJB*65] bf16
        vto = ps.tile([128, NJB * 65], BF16, name="vto", tag="vto")
        ones_ap = vto[:].rearrange("p (g e) -> p g e", e=65)[:, :, 64:65]
        nc.vector.memset(ones_ap, 1.0)

        def emit_vt(jb):
            s, col = jb // 4, (jb % 4) * 128
            vp = acc.tile([128, ICH], F32, name="acc", tag="acc")
            for cb in range(2):
                nc.tensor.matmul(vp[:, 0:D], xb[cb][s][:, col:col + 128], wvT[cb][:],
                                 start=(cb == 0), stop=(cb == 1))
            nc.vector.tensor_tensor(vto[:, jb * 65:jb * 65 + 64], vp[:, 0:D],
                                    bvB[:], op=OP.add)

        emit_qq(0)
        if reps > 1:
            # timing build: everything hoisted out of the repeat loop
            for ic in range(1, NIC):
                emit_qq(ic)
            for jb in range(NJB):
                emit_vt(jb)

        # --- main attention loop ---
        rep_ctx = tc.For_i(0, reps, 1) if reps > 1 else None
        if rep_ctx is not None:
            ctx.enter_context(rep_ctx)
        groups = [list(range(g, min(g + GJ, NJB))) for g in range(0, NJB, GJ)]

        if variant in ("expo", "umm"):
            # microbenchmarks on static tiles
            sc0 = scp.tile([128, GJ * ICH], F32, name="sc0", tag="sc")
            nc.vector.memset(sc0[:], 0.0)
            ex0 = xt.tile([128, GJ * ICH], BF16, name="ex0", tag="xt")
            nc.vector.memset(ex0[:], 0.25)
            for ic in range(NIC):
                if variant == "expo":
                    for jbs in groups:
                        ex = xt.tile([128, GJ * ICH], BF16, name="xt", tag="xt")
                        nc.scalar.activation(ex[:, 0:len(jbs) * ICH],
                                             sc0[:, 0:len(jbs) * ICH],
                                             AF.Exp, bias=nshift[:])
                else:
                    U = acc.tile([128, ICH], F32, name="acc", tag="acc")
                    for jb in range(NJB):
                        nc.tensor.matmul(U[0:65, :], vto[:, jb * 65:jb * 65 + 65],
                                         ex0[:, (jb % GJ) * ICH:(jb % GJ + 1) * ICH],
                                         start=(jb == 0), stop=(jb == NJB - 1))
                    nc.vector.tensor_copy(rcdrain := ep.tile(
                        [65, ICH], F32, name="u_sb", tag="u_sb"), U[0:65, :])
        elif variant in ("scores", "scores_unpaired"):
            for ic in range(NIC):
                for gi, jbs in enumerate(groups):
                    sc = scp.tile([128, GJ * ICH], F32, name="sc", tag="sc")
                    for t, jb in enumerate(jbs):
                        s, col = jb // 4, (jb % 4) * 128
                        rg = (jb % 2) if variant == "scores" else 0
                        nc.tensor.matmul(sc[:, t * ICH:(t + 1) * ICH],
                                         kk[s][rg * D:(rg + 1) * D, col:col + 128],
                                         qq[ic][rg * D:(rg + 1) * D, :],
                                         start=True, stop=True)

        if variant != "full":
            groups = []

        # Software-pipelined emission: each group's U matmuls are emitted
        # AFTER the next group's scores matmuls so the PE FIFO never blocks
        # the ACT (exp) feed. The epilogue is likewise spread as small steps
        # across the next chunk's groups.
        pend = None         # (U, ex, jbs) — U matmuls not yet emitted
        epi_steps = []      # staged epilogue closures from the previous chunk

        def flush_u():
            nonlocal pend
            if pend is None:
                return
            Uh, exh, jbs_p = pend
            pend = None
            for t, jb in enumerate(jbs_p):
                nc.tensor.matmul(Uh[0:65, :], vto[:, jb * 65:jb * 65 + 65],
                                 exh[:, t * ICH:(t + 1) * ICH],
                                 start=(jb == 0), stop=(jb == NJB - 1))

        def make_epilogue(Uh, ic):
            state = {}

            def s1():
                u_sb = ep.tile([65, ICH], F32, name="u_sb", tag="u_sb")
                nc.vector.tensor_copy(u_sb[:], Uh[0:65, :])
                rc65 = ep.tile([65, ICH], F32, name="rc65", tag="rc65")
                nc.vector.reciprocal(rc65[D:D + 1, :], u_sb[D:D + 1, :])
                rcb_bf = ep.tile([65, ICH], BF16, name="rcb_bf", tag="rcb_bf")
                nc.vector.tensor_copy(rcb_bf[D:D + 1, :], rc65[D:D + 1, :])
                state.update(u_sb=u_sb, rcb_bf=rcb_bf)

            def s2():
                rcb = acc.tile([128, ICH], F32, name="rcb", tag="acc")
                nc.tensor.matmul(rcb[0:D, :], ones_bf65[D:D + 1, 0:D],
                                 state["rcb_bf"][D:D + 1, :], start=True, stop=True)
                usb = ep.tile([65, ICH], BF16, name="usb", tag="usb")
                nc.vector.tensor_tensor(usb[0:D, :], state["u_sb"][0:D, :],
                                        rcb[0:D, :], op=OP.mult)
                nc.vector.memset(usb[D:D + 1, :], 1.0)
                state["usb"] = usb

            def proj(cb):
                op_ps = acc.tile([128, ICH], F32, name="acc", tag="acc")
                nc.tensor.matmul(op_ps[:], wsaT[:, cb * 128:(cb + 1) * 128],
                                 state["usb"][:], start=True, stop=True)
                fin = ep.tile([128, ICH], F32, name="fin", tag="fin")
                nc.vector.scalar_tensor_tensor(
                    out=fin[:], in0=op_ps[:], scalar=gammaB[:], in1=xf[cb][ic][:],
                    op0=OP.mult, op1=OP.add)
                nc.sync.dma_start(
                    out_d[cb * 128:(cb + 1) * 128, ic * ICH:(ic + 1) * ICH], fin[:])

            return [s1, s2, lambda: proj(0), lambda: proj(1)]

        for ic in range(NIC if variant == "full" else 0):
            U = acc.tile([128, ICH], F32, name="acc", tag="acc")
            for gi, jbs in enumerate(groups):
                sc = scp.tile([128, GJ * ICH], F32, name="sc", tag="sc")
                for t, jb in enumerate(jbs):
                    s, col = jb // 4, (jb % 4) * 128
                    rg = jb % 2  # alternate PE row groups -> pairs overlap
                    nc.tensor.matmul(sc[:, t * ICH:(t + 1) * ICH],
                                     kk[s][rg * D:(rg + 1) * D, col:col + 128],
                                     qq[ic][rg * D:(rg + 1) * D, :],
                                     start=True, stop=True)
                flush_u()
                if epi_steps:
                    epi_steps.pop(0)()
                ex = xt.tile([128, GJ * ICH], BF16, name="xt", tag="xt")
                n = len(jbs) * ICH
                nc.scalar.activation(ex[:, 0:n], sc[:, 0:n], AF.Exp, bias=nshift[:])
                if ic == 0 and reps == 1:
                    # first pass: produce v^T blocks just-in-time (fills PE
                    # while ACT works) and later q chunks early
                    for jb in jbs:
                        emit_vt(jb)
                    if gi < NIC - 1:
                        emit_qq(gi + 1)
                pend = (U, ex, jbs)
            epi_steps = make_epilogue(U, ic)
        if variant == "full":
            flush_u()
            for step in epi_steps:
                step()

    nc.compile()
    return nc


_CACHE = {}


def _get_program():
    if "nc" not in _CACHE:
        _CACHE["nc"] = build_program()
    return _CACHE["nc"]


def make_in_maps(inputs):
    x = np.asarray(inputs["x"], np.float32).reshape(B, C, HW)
    wq = np.ascontiguousarray(np.asarray(inputs["wq"], np.float32))
    wk = np.ascontiguousarray(np.asarray(inputs["wk"], np.float32))
    wv = np.ascontiguousarray(np.asarray(inputs["wv"], np.float32))
    wsa = np.ascontiguousarray(np.asarray(inputs["wsa"], np.float32))
    bq = np.asarray(inputs["bq"], np.float32).reshape(D, 1)
    bk = np.asarray(inputs["bk"], np.float32).reshape(D, 1)
    bv = np.asarray(inputs["bv"], np.float32).reshape(1, D)
    bsa = np.asarray(inputs["bsa"], np.float32).reshape(1, C)
    gamma = np.asarray(inputs["gamma"], np.float32).reshape(1, 1)

    in_maps = []
    for core in range(N_CORES):
        b, h = core // 2, core % 2
        if h == 0:
            xc = x[b]
        else:
            xc = np.concatenate([x[b][:, IH:], x[b][:, :IH]], axis=1)
        in_maps.append({
            "x": np.ascontiguousarray(xc),
            "wq": wq, "wk": wk, "wv": wv, "wsa": wsa,
            "bq": bq, "bk": bk, "bv": bv, "bsa": bsa, "gamma": gamma,
        })
    return in_maps


def assemble_output(results):
    out = np.empty((B, C, HW), np.float32)
    for core in range(N_CORES):
        b, h = core // 2, core % 2
        out[b][:, h * IH:(h + 1) * IH] = results[core]["out"]
    return out.reshape(B, C, 64, 64)


def kernel(**inputs):
    nc = _get_program()
    in_maps = make_in_maps(inputs)
    res = run_bass_kernel_spmd(nc, in_maps, core_ids=list(range(N_CORES)))
    return assemble_output(res.results)
